# revision 21
# baseline (speedup 1.0000x reference)
"""CodebookLinear TRN2 kernel.

Reference computation (jax):
    W = codebook[indices].reshape(-1)[:4096*4096].reshape(4096, 4096)   # [out, in]
    out = einsum('bsi,oi->bso', x, W) + bias

Distribution: 8 NeuronCores, column-parallel over out_features (each core
owns 512 output features and all 8192 tokens), no collectives.

Per-core structure (the codebook gather is the hard part: any per-index
gather primitive on TRN2 costs ~25 ns/index because Cayman RD_CMDs do not
pipeline, so the 32768-index gather is ~0.88 ms of Pool-engine time; the
kernel hides most of the matmul work under it):

  setup:    PE-transposes the codebook so SBUF partition p holds codebook
            column k(p)  -> data[128, 4096].
  gather:   reconstructs W^T [i, o] in bf16 in SBUF in four K-quarters via
            GpSimd ap_gather: per 16-partition group the index list
            interleaves the two j-block columns owned by that group; a
            stride-2-free tensor_copy + copy_predicated (per-partition
            parity mask) selects/casts into the resident W^T quarter.
  mm q0-q2: as soon as K-quarter q's W^T is resident, a matmul pass over
            all 64 token tiles runs UNDER the remaining gathers:
            x^T via HWDGE f32 loads (SP engine) + casts and PSUM drains on
            the Scalar engine — deliberately avoiding both the Pool queue
            and the DVE<->GpSimd shared SBUF port so the gathers are not
            slowed. Quarter partials go to DRAM (bias preloaded into PSUM
            in q0 via a K=1 f32 matmul).
  mm final: after the last gather the Pool/DVE engines are free: x loads
            switch to SWDGE f32->bf16 cast-DMA, the three DRAM partials
            are loaded and summed on DVE, and rows are stored.

Host side only shards/reshapes: x is passed transposed and row-permuted
(layout choice), indices are converted to int16 and pre-permuted into the
wrapped per-group interleaved layout the gather consumes (pure
permutation), bias is sliced.  Output is assembled to [4, 2048, 4096].

Index/partition math (per core, o local in [0, O_LOC)):
  Within k-tile it, SBUF partition p holds contraction row
      i = 128*it + sigma(p),  sigma(p) = 8*(2*(p>>4) + (p&1)) + ((p>>1)&7)
  so  j(i) = 16*it + 2*g + h,  k(i) = (p>>1)&7,  g = p>>4,  h = p&1.
  group g's list for k-tile it:  L[n = 2*o + h] = idx[o, 16*it + 2*g + h]
  wrapped storage:               idxw[16*g + q, it, f] = L[16*f + q]
  gather:  g2[p, n] = data[p, L[g(p)][n]] = cb[idx[o(n), j], k(p)]
  select:  W^T[p, o] = g2[p, 2*o + (p&1)]

Measured on 8 axon TRN2 cores: HW exec ~1.26 ms, rel err 2.35e-03 (bf16).
"""

import sys

for _p in ("/opt/trn_rl_repo",):
    if _p not in sys.path:
        sys.path.insert(0, _p)

import numpy as np

import concourse.bacc as bacc
import concourse.mybir as mybir
import concourse.tile as tile
from concourse.bass_utils import run_bass_kernel_spmd
from concourse.masks import make_identity

# Problem constants
OUT_F = 4096
IN_F = 4096
KCB = 4096          # codebook entries
BS = 8              # block size
JB = IN_F // BS     # 512 blocks per W row
B, S = 4, 2048
T = B * S           # 8192 tokens

# Shard grid: S_O x S_T = 8 cores
S_O, S_T = 8, 1
O_LOC = OUT_F // S_O   # 1024
T_LOC = T // S_T       # 4096

P = 128
NIT = IN_F // P        # 32 k-tiles
NTT = T_LOC // P       # 32 token tiles
NOH = 1                # whole o-shard fits one PSUM pass (512)

# partition -> within-tile contraction row
_p_ar = np.arange(P)
SIGMA = (8 * (2 * (_p_ar >> 4) + (_p_ar & 1)) + ((_p_ar >> 1) & 7)).astype(np.int64)

_nc_cache = None
last_result = None     # BassKernelResults of the most recent run (for test.py)


def build_nc():
    nc = bacc.Bacc("TRN2", target_bir_lowering=False, debug=False)
    xT = nc.dram_tensor("xT", [IN_F, T_LOC], mybir.dt.float32, kind="ExternalInput")
    idxw = nc.dram_tensor("idxw", [P, NIT * (2 * O_LOC // 16)], mybir.dt.int16, kind="ExternalInput")
    cb = nc.dram_tensor("cb", [KCB, BS], mybir.dt.float32, kind="ExternalInput")
    bias = nc.dram_tensor("bias", [1, O_LOC], mybir.dt.float32, kind="ExternalInput")
    mask = nc.dram_tensor("mask", [P, 1], mybir.dt.uint8, kind="ExternalInput")
    out = nc.dram_tensor("out", [T_LOC, O_LOC], mybir.dt.float32, kind="ExternalOutput")
    cbt16_dram = nc.dram_tensor("cbt16_scratch", [16, KCB], mybir.dt.float32)
    out_parts = [
        nc.dram_tensor(f"out_partial{q}", [T_LOC, O_LOC], mybir.dt.float32)
        for q in range(3)
    ]

    with tile.TileContext(nc) as tc:
        with (
            tc.tile_pool(name="const", bufs=1) as constp,
            tc.tile_pool(name="wt", bufs=1) as wtp,
            tc.tile_pool(name="g2p", bufs=2) as g2p,
            tc.tile_pool(name="xfp", bufs=3) as xfp,
            tc.tile_pool(name="xbp", bufs=4) as xbp,
            tc.tile_pool(name="outp", bufs=2) as outp,
            tc.tile_pool(name="psmm", bufs=4, space="PSUM") as psmm,
            tc.tile_pool(name="pstr", bufs=2, space="PSUM") as pstr,
        ):
            identity = constp.tile([P, P], mybir.dt.float32)
            make_identity(nc, identity[:])
            ones_row = constp.tile([1, P], mybir.dt.float32)
            nc.gpsimd.memset(ones_row[:], 1.0)
            bias_row = constp.tile([1, O_LOC], mybir.dt.float32)
            nc.sync.dma_start(out=bias_row[:], in_=bias[:, :])
            mask_t = constp.tile([P, 1], mybir.dt.uint8)
            nc.sync.dma_start(out=mask_t[:], in_=mask[:, :])

            # ---- setup: cb^T, duplicated pairwise, replicated to 128 parts ----
            cbn = constp.tile([P, NIT * BS], mybir.dt.float32)  # cb rows on partitions
            nc.sync.dma_start(
                out=cbn[:].rearrange("p (a b) -> p a b", b=BS),
                in_=cb[:, :].rearrange("(a p) b -> p a b", p=P),
            )
            cbt = constp.tile([BS, KCB], mybir.dt.float32)
            for a in range(NIT):
                pst = pstr.tile([BS, P], mybir.dt.float32)
                nc.tensor.transpose(
                    out=pst[:],
                    in_=cbn[:, a * BS : (a + 1) * BS],
                    identity=identity[:],
                )
                nc.vector.tensor_copy(out=cbt[:, a * P : (a + 1) * P], in_=pst[:])
            # cbt16[2k + h] = cbt[k]
            for h in range(2):
                nc.sync.dma_start(
                    out=cbt16_dram[:, :].rearrange("(a h) f -> a h f", h=2)[:, h],
                    in_=cbt[:],
                )
            data = constp.tile([P, KCB], mybir.dt.float32)
            for g in range(8):
                nc.sync.dma_start(
                    out=data[16 * g : 16 * (g + 1), :], in_=cbt16_dram[:, :]
                )

            # indices, pre-wrapped on host
            idxt = constp.tile([P, NIT * (2 * O_LOC // 16)], mybir.dt.int16)
            nc.sync.dma_start(out=idxt[:], in_=idxw[:, :])

            # Resident W^T, bf16, in four K-quarters (k-split overlap): [p, kt, o]
            NQ = 4
            KQ = NIT // NQ
            WTk = [
                wtp.tile([P, KQ, O_LOC], mybir.dt.bfloat16, name=f"WTk{k}")
                for k in range(NQ)
            ]

            FW = 2 * O_LOC // 16   # wrapped index columns per k-tile
            mask_bc = mask_t[:, 0:1].to_broadcast([P, O_LOC])
            xTr = xT[:, :].rearrange("(it p) t -> p it t", p=P)  # [128, NIT, T_LOC]

            def gather_kq(kh):
                # Pool engine only (+ small DVE selects)
                for itl in range(KQ):
                    it = kh * KQ + itl
                    g2 = g2p.tile([P, 2 * O_LOC], mybir.dt.float32)
                    nc.gpsimd.ap_gather(
                        out_ap=g2[:, :],
                        in_ap=data[:, :],
                        idxs_ap=idxt[:, it * FW : (it + 1) * FW],
                        channels=P,
                        num_elems=KCB,
                        d=1,
                        num_idxs=2 * O_LOC,
                    )
                    g2_s = g2[:, :].rearrange("p (o s) -> p o s", s=2)
                    nc.vector.tensor_copy(out=WTk[kh][:, itl, :], in_=g2_s[:, :, 0])
                    nc.vector.copy_predicated(
                        out=WTk[kh][:, itl, :], mask=mask_bc, data=g2_s[:, :, 1]
                    )

            def mm_pass_under(q):
                # quarters 0..2 run under later gathers: x via HWDGE f32
                # (SP engine), cast + PSUM drain on the Scalar engine —
                # no Pool-queue or DVE-port interaction with the gathers.
                for tt2 in range(NTT // 2):
                    xf = xfp.tile([P, KQ, 2 * P], mybir.dt.float32, name="xf")
                    nc.sync.dma_start(
                        out=xf[:, :, :],
                        in_=xTr[:, q * KQ : (q + 1) * KQ,
                                tt2 * 2 * P : (tt2 + 1) * 2 * P],
                    )
                    xt2 = xbp.tile([P, KQ, 2 * P], mybir.dt.bfloat16, name="xt2")
                    nc.scalar.copy(out=xt2[:, :, :], in_=xf[:, :, :])
                    for u in range(2):
                        tt = tt2 * 2 + u
                        outt = outp.tile([P, O_LOC], mybir.dt.float32)
                        ps = psmm.tile([P, O_LOC], mybir.dt.float32)
                        if q == 0:
                            # bias preload: psum[t, o] = ones[t] * bias[o]
                            nc.tensor.matmul(
                                out=ps[:],
                                lhsT=ones_row[:, :],
                                rhs=bias_row[:, :],
                                start=True,
                                stop=False,
                            )
                        for itl in range(KQ):
                            nc.tensor.matmul(
                                out=ps[:],
                                lhsT=xt2[:, itl, u * P : (u + 1) * P],
                                rhs=WTk[q][:, itl, :],
                                start=(q != 0 and itl == 0),
                                stop=(itl == KQ - 1),
                            )
                        nc.scalar.copy(out=outt[:, :], in_=ps[:])
                        nc.sync.dma_start(
                            out=out_parts[q][tt * P : (tt + 1) * P, :],
                            in_=outt[:],
                        )

            def mm_pass_final():
                # last quarter after all gathers: Pool free -> SWDGE
                # cast-DMA loads; DVE free -> sum the three partials.
                for tt in range(NTT):
                    xt = xbp.tile([P, KQ, P], mybir.dt.bfloat16, name="xtb")
                    nc.gpsimd.dma_start(
                        out=xt[:, :, :],
                        in_=xTr[:, 3 * KQ : NIT, tt * P : (tt + 1) * P],
                    )
                    parts = []
                    for q in range(3):
                        pt = outp.tile([P, O_LOC], mybir.dt.float32, name=f"part{q}")
                        nc.sync.dma_start(
                            out=pt[:], in_=out_parts[q][tt * P : (tt + 1) * P, :]
                        )
                        parts.append(pt)
                    outt = outp.tile([P, O_LOC], mybir.dt.float32)
                    ps = psmm.tile([P, O_LOC], mybir.dt.float32)
                    for itl in range(KQ):
                        nc.tensor.matmul(
                            out=ps[:],
                            lhsT=xt[:, itl, :],
                            rhs=WTk[3][:, itl, :],
                            start=(itl == 0),
                            stop=(itl == KQ - 1),
                        )
                    nc.vector.tensor_tensor(
                        out=outt[:, :], in0=ps[:], in1=parts[0][:],
                        op=mybir.AluOpType.add,
                    )
                    nc.vector.tensor_tensor(
                        out=outt[:, :], in0=outt[:, :], in1=parts[1][:],
                        op=mybir.AluOpType.add,
                    )
                    nc.vector.tensor_tensor(
                        out=outt[:, :], in0=outt[:, :], in1=parts[2][:],
                        op=mybir.AluOpType.add,
                    )
                    nc.sync.dma_start(
                        out=out[tt * P : (tt + 1) * P, :], in_=outt[:]
                    )

            gather_kq(0)
            mm_pass_under(0)
            gather_kq(1)
            mm_pass_under(1)
            gather_kq(2)
            mm_pass_under(2)
            gather_kq(3)
            mm_pass_final()

    nc.compile()
    return nc


def _get_nc():
    global _nc_cache
    if _nc_cache is None:
        _nc_cache = build_nc()
    return _nc_cache


def _wrap_indices(idx_local):
    """[O_LOC, JB] int -> wrapped interleaved uint16 [P, NIT*P]."""
    arr = idx_local.reshape(O_LOC, NIT, 8, 2)        # [o, it, g, h]
    L = arr.transpose(2, 1, 0, 3).reshape(8, NIT, 2 * O_LOC)   # [g, it, n=2o+h]
    Lw = L.reshape(8, NIT, 2 * O_LOC // 16, 16)      # [g, it, f, q]
    idxw = Lw.transpose(0, 3, 1, 2).reshape(P, NIT * (2 * O_LOC // 16))
    return np.ascontiguousarray(idxw.astype(np.int16))


def make_in_maps(x, codebook, indices, bias):
    x = np.asarray(x, dtype=np.float32).reshape(T, IN_F)
    xT_full = np.ascontiguousarray(x.T)  # [IN_F, T]
    # permute contraction rows within each 128-tile to match the W^T layout
    xT_perm = np.ascontiguousarray(
        xT_full.reshape(NIT, P, T)[:, SIGMA, :].reshape(IN_F, T)
    )
    idx2d = np.asarray(indices).astype(np.int64).reshape(OUT_F, JB)
    cb = np.ascontiguousarray(np.asarray(codebook, dtype=np.float32))
    b = np.asarray(bias, dtype=np.float32)
    mask_np = (np.arange(P) % 2).astype(np.uint8).reshape(P, 1)

    in_maps = []
    for c in range(8):
        ot, tt = c % S_O, c // S_O
        in_maps.append(
            {
                "xT": np.ascontiguousarray(xT_perm[:, tt * T_LOC : (tt + 1) * T_LOC]),
                "idxw": _wrap_indices(idx2d[ot * O_LOC : (ot + 1) * O_LOC]),
                "cb": cb,
                "bias": np.ascontiguousarray(
                    b[ot * O_LOC : (ot + 1) * O_LOC]
                ).reshape(1, O_LOC),
                "mask": mask_np,
            }
        )
    return in_maps


def assemble(outs):
    full = np.empty((T, OUT_F), dtype=np.float32)
    for c in range(8):
        ot, tt = c % S_O, c // S_O
        full[tt * T_LOC : (tt + 1) * T_LOC, ot * O_LOC : (ot + 1) * O_LOC] = outs[c][
            "out"
        ]
    return full.reshape(B, S, OUT_F)


def kernel(x, codebook, indices, bias):
    global last_result
    nc = _get_nc()
    in_maps = make_in_maps(x, codebook, indices, bias)
    last_result = run_bass_kernel_spmd(nc, in_maps, core_ids=list(range(8)))
    return assemble(last_result.results)
